# revision 13
# baseline (speedup 1.0000x reference)
"""Trainium2 Bass kernel for nn_LocalEnergy (protein local-energy GNN), v3.

kernel(**inputs) takes FULL unsharded inputs (B=128), shards B across 8
NeuronCores (16 samples/core, pure data parallel), runs one Bass kernel
SPMD, gathers per-core [16] energies into the full [128] output.

v3 design (vs v2 baseline at 177us):
 - fp8 DoubleRow matmuls for W1 and W2 (0.5 PE cycles/row): the per-sample
   input stack is host-laid-out as [35, 2, L] (two fp8 k-tiles: emb shifts
   0/1 + len row in ktile0, shifts 2/3 + sin/cos/cos_t rows in ktile1).
   h1r is refolded [128,N] -> [64,2,N] by one SBUF->SBUF DMA per unit (W2
   rows host-interleaved to match the DMA's q -> (q//2, q%2) mapping).
 - b1/b2 biases applied inside the relu passes (no ones row; K stays 70).
 - relu / relu+accum passes distributed across ALL THREE elementwise
   engines (ACT activation, DVE & Pool scalar_tensor_tensor with accum),
   chosen by a greedy load-balancer.
 - geometry: one act-table (sqrt/square/relu), shifted views instead of
   DMA copies, rotation via a single PE matmul family, ALU divide for
   sin/cos normalization.
"""

import sys
import types
import numpy as np
import ml_dtypes
from contextlib import ExitStack


def ensure_axon_hooks():
    """The container's antenv is a stub without axon_hooks; inject it so
    run_bass_kernel_spmd(trace=True) can NTFF-profile."""
    if "antenv.axon_hooks" in sys.modules:
        return
    import antenv

    hooks = types.ModuleType("antenv.axon_hooks")
    hooks._h = None

    def set_axon_ntff_profile_hook(h):
        hooks._h = h

    def get_axon_ntff_profile_hook():
        return hooks._h

    hooks.set_axon_ntff_profile_hook = set_axon_ntff_profile_hook
    hooks.get_axon_ntff_profile_hook = get_axon_ntff_profile_hook
    sys.modules["antenv.axon_hooks"] = hooks
    antenv.axon_hooks = hooks
    try:
        from trn_agent_boot.trn_boot import _ntff_profile_via_ctypes

        hook = _ntff_profile_via_ctypes("/opt/axon/libaxon_pjrt.so")
        if hook is not None:
            set_axon_ntff_profile_hook(hook)
    except Exception:
        pass


ensure_axon_hooks()

import concourse.bass as bass  # noqa: E402
import concourse.tile as tile  # noqa: E402
from concourse import mybir, bacc, bass_utils  # noqa: E402

dt = mybir.dt
AF = mybir.ActivationFunctionType
ALU = mybir.AluOpType
AX = mybir.AxisListType
PM = mybir.MatmulPerfMode
FP8 = ml_dtypes.float8_e4m3

NCORES = 8
B, L, NAA, E, H = 128, 2048, 20, 16, 128
BPC = B // NCORES
MLPS = ("fl", "ft", "fp")
KOFF = (1, 2, 3)                  # valid cols per sample = L - KOFF[m]
SINV = 1.0 / 16.0                 # bond-vector scaling to stay in fp16 range
NL, NT, NP = L - 1, L - 2, L - 3

# per-unit engine cost estimates (us) used by the greedy scheduler:
# (relu unit, relu+accum unit); preload = geometry work per engine.
# Only ACT and DVE can access PSUM (Pool/gpsimd is SBUF-only on TRN2).
ECOST = {"A": (1.07, 1.35), "D": (1.26, 1.26)}
EPRELOAD = {"A": 11.0, "D": 18.0}
LAG = 3                           # units between W1 emission and W2 emission


def plan_engines():
    units = [(s, mi, h) for s in range(BPC) for mi in range(3) for h in range(2)]
    load = dict(EPRELOAD)
    e1, e2 = [], []
    for _ in units:
        e = min("AD", key=lambda k: load[k] + ECOST[k][0])
        e1.append(e)
        load[e] += ECOST[e][0]
        e = min("AD", key=lambda k: load[k] + ECOST[k][1])
        e2.append(e)
        load[e] += ECOST[e][1]
    return units, e1, e2


def build_nc(bpc=BPC, ll=L):
    nc = bacc.Bacc("TRN2", target_bir_lowering=False, debug=False)

    Rt_d = nc.dram_tensor("Rt", (3, bpc, ll), dt.float32, kind="ExternalInput")
    SE_d = nc.dram_tensor("SE", (bpc, 32, 2, ll), dt.float8e4, kind="ExternalInput")
    SEL_d = nc.dram_tensor("SEL", (48, 128), dt.float16, kind="ExternalInput")
    WD1_d = nc.dram_tensor("WD1", (35, 3, 2, H), dt.float8e4, kind="ExternalInput")
    WD2_d = nc.dram_tensor("WD2", (H, 3, H), dt.float16, kind="ExternalInput")
    BW_d = nc.dram_tensor("BW", (H, 10), dt.float32, kind="ExternalInput")
    out_d = nc.dram_tensor("out", (1, bpc), dt.float32, kind="ExternalOutput")

    nl, nt, np_ = ll - 1, ll - 2, ll - 3
    units, E1, E2 = plan_engines()

    def veng(e):
        return {"D": nc.vector, "P": nc.gpsimd}[e]

    with tile.TileContext(nc) as tc, ExitStack() as ctx:
        consts = ctx.enter_context(tc.tile_pool(name="consts", bufs=1))

        rt = consts.tile([48, ll], dt.float32, name="rt")
        for c in range(3):
            nc.sync.dma_start(out=rt[16 * c: 16 * c + bpc, :], in_=Rt_d.ap()[c])
        sel = consts.tile([48, 128], dt.float16, name="sel")
        nc.sync.dma_start(out=sel, in_=SEL_d.ap())
        srot, s48 = sel[:, 0:112], sel[:, 112:128]
        wd1 = consts.tile([35, 3, 2, H], dt.float8e4, name="wd1")
        nc.sync.dma_start(out=wd1, in_=WD1_d.ap())
        wd2 = consts.tile([H, 3, H], dt.float16, name="wd2")
        nc.sync.dma_start(out=wd2, in_=WD2_d.ap())
        bw = consts.tile([H, 10], dt.float32, name="bw")
        nc.sync.dma_start(out=bw, in_=BW_d.ap())

        # persistent per-sample stacks [35, s, ktile, L] fp8
        stacks = consts.tile([35, bpc, 2, ll], dt.float8e4, name="stacks")
        for s in range(bpc):
            nc.sync.dma_start(out=stacks[0:32, s], in_=SE_d.ap()[s])

        # feature tile [sample, j, L]; j: 0=len-3.8, 1=sin', 2=zero, 3=cosphi,
        # 4=zero, 5=cost -- j-order matches the stack slots (32,k0..34,k1)
        F = consts.tile([16, 6, ll], dt.float8e4, name="F")
        nc.gpsimd.memset(F[:, 2, :], 0.0)
        nc.gpsimd.memset(F[:, 4, :], 0.0)
        nc.gpsimd.memset(F[:, :, ll - 8: ll], 0.0)
        eps_t = consts.tile([16, 1], dt.float32, name="eps_t")
        nc.vector.memset(eps_t, 1e-6)
        zeros = consts.tile([H, 1024], dt.float16, name="zeros")
        nc.vector.memset(zeros, 0.0)
        scr = {}
        for e in "ADP":
            scr[e] = consts.tile([H, 1024], dt.float16, name=f"scr{e}")
        acc = consts.tile([H, 96], dt.float32, name="acc")

        # ---------------- Phase 1: geometry ----------------
        with tc.tile_pool(name="geo", bufs=1) as geo, \
             tc.tile_pool(name="geo_ps", bufs=2, space="PSUM") as geo_ps:

            D = geo.tile([48, nl], dt.float16, name="D")
            nc.gpsimd.tensor_tensor(out=D, in0=rt[:, 1:ll], in1=rt[:, 0:nl], op=ALU.subtract)
            Ds = geo.tile([48, nl], dt.float16, name="Ds")
            nc.vector.tensor_scalar(out=Ds, in0=D, scalar1=SINV, scalar2=None, op0=ALU.mult)
            DSQ = geo.tile([48, nl], dt.float16, name="DSQ")
            nc.vector.tensor_tensor(out=DSQ, in0=D, in1=D, op=ALU.mult)
            DD = geo.tile([48, nt], dt.float16, name="DD")
            nc.vector.tensor_tensor(out=DD, in0=D[:, 0:nt], in1=D[:, 1:nl], op=ALU.mult)

            def chunked_mm(dst_flat, lhsT, src, count):
                for c0 in range(0, count, 512):
                    n = min(512, count - c0)
                    nc.tensor.matmul(dst_flat[:, c0:c0 + n], lhsT, src[:, c0:c0 + n],
                                     start=True, stop=True)

            # all per-sample scalars go to psum partition base 0 (one tile each)
            ld1 = geo_ps.tile([128, 4, 512], dt.float32, name="ld1", tag="gps")
            ld1f = ld1.rearrange("p a b -> p (a b)")
            chunked_mm(ld1f[0:16, :], s48, DSQ, nl)               # lsq
            lent = geo.tile([16, nl], dt.float16, name="lent")
            nc.scalar.activation(out=lent, in_=ld1f[0:16, 0:nl], func=AF.Sqrt)
            nc.gpsimd.tensor_scalar(out=F[:, 0, 0:nl], in0=lent, scalar1=3.8,
                                    scalar2=None, op0=ALU.subtract)

            ld2 = geo_ps.tile([128, 4, 512], dt.float32, name="ld2", tag="gps")
            ld2f = ld2.rearrange("p a b -> p (a b)")
            chunked_mm(ld2f[0:16, :], s48, DD, nt)                # dot
            lenp32 = geo.tile([16, nt], dt.float32, name="lenp32")
            nc.gpsimd.tensor_tensor(out=lenp32, in0=lent[:, 0:nt], in1=lent[:, 1:nl], op=ALU.mult)
            rlenp32 = geo.tile([16, nt], dt.float32, name="rlenp32")
            nc.vector.reciprocal_approx_fast(out=rlenp32, in_=lenp32)
            # -cos(theta) = +dot/(len*len1); sign folded into W1
            nc.vector.tensor_tensor(out=F[:, 5, 0:nt], in0=ld2f[0:16, 0:nt],
                                    in1=rlenp32, op=ALU.mult)

            # rotations A1@0:48, A2@64:112 via one matmul family
            pa = geo_ps.tile([128, 4, 512], dt.float32, name="pa", tag="gps")
            paf = pa.rearrange("p a b -> p (a b)")
            chunked_mm(paf[0:112, :], srot, Ds, nl)
            a1sb = geo.tile([48, nl], dt.float16, name="a1sb")
            nc.scalar.activation(out=a1sb, in_=paf[0:48, 0:nl], func=AF.Copy)
            a2sb = geo.tile([48, nl], dt.float16, name="a2sb")
            nc.scalar.activation(out=a2sb, in_=paf[64:112, 0:nl], func=AF.Copy)
            t_a = geo.tile([48, nt], dt.float16, name="t_a")
            nc.vector.tensor_tensor(out=t_a, in0=a1sb[:, 0:nt], in1=a2sb[:, 1:nl], op=ALU.mult)
            t_b = geo.tile([48, nt], dt.float16, name="t_b")
            nc.vector.tensor_tensor(out=t_b, in0=a2sb[:, 0:nt], in1=a1sb[:, 1:nl], op=ALU.mult)
            Cs = geo.tile([48, nt], dt.float16, name="Cs")
            nc.vector.tensor_tensor(out=Cs, in0=t_a, in1=t_b, op=ALU.subtract)
            XRp = geo.tile([48, np_], dt.float16, name="XRp")
            nc.vector.tensor_tensor(out=XRp, in0=Cs[:, 0:np_], in1=Cs[:, 1:nt], op=ALU.mult)
            YRp = geo.tile([48, np_], dt.float16, name="YRp")
            nc.gpsimd.tensor_tensor(out=YRp, in0=Ds[:, 0:np_], in1=Cs[:, 1:nt], op=ALU.mult)

            xyx = geo_ps.tile([128, 4, 512], dt.float32, name="xyx", tag="gps")
            xyxf = xyx.rearrange("p a b -> p (a b)")
            chunked_mm(xyxf[0:16, :], s48, XRp, np_)              # x = n1.n2
            xx = geo.tile([16, np_], dt.float16, name="xx")
            nc.scalar.activation(out=xx, in_=xyxf[0:16, 0:np_], func=AF.Square)

            xyy = geo_ps.tile([128, 4, 512], dt.float32, name="xyy", tag="gps")
            xyyf = xyy.rearrange("p a b -> p (a b)")
            chunked_mm(xyyf[0:16, :], s48, YRp, np_)              # y-raw
            y_t = geo.tile([16, np_], dt.float16, name="y_t")
            nc.vector.scalar_tensor_tensor(out=y_t, in0=xyyf[0:16, 0:np_], scalar=SINV,
                                           in1=lent[:, 1:1 + np_], op0=ALU.mult, op1=ALU.mult)
            yy = geo.tile([16, np_], dt.float16, name="yy")
            nc.gpsimd.tensor_tensor(out=yy, in0=y_t, in1=y_t, op=ALU.mult)
            q = geo.tile([16, np_], dt.float16, name="q")
            nc.gpsimd.tensor_tensor(out=q, in0=xx, in1=yy, op=ALU.add)
            sq32 = geo.tile([16, np_], dt.float32, name="sq32")
            nc.scalar.activation(out=sq32, in_=q, func=AF.Sqrt, bias=eps_t)
            rsq32 = geo.tile([16, np_], dt.float32, name="rsq32")
            nc.vector.reciprocal_approx_fast(out=rsq32, in_=sq32)
            # -sin(phi) convention: stored y/|..| matches v2's y*r2; W1 row = -w
            nc.gpsimd.tensor_tensor(out=F[:, 1, 0:np_], in0=y_t, in1=rsq32, op=ALU.mult)
            nc.vector.tensor_tensor(out=F[:, 3, 0:np_], in0=xyxf[0:16, 0:np_], in1=rsq32, op=ALU.mult)

        # per-sample feature DMA into the stacks (6 slots -> rows 32:35 x 2 ktiles)
        for s in range(bpc):
            nc.sync.dma_start(out=stacks[32:35, s, :, :], in_=F[s:s + 1, :, :])

        # ---------------- Phase 2: MLP units ----------------
        with tc.tile_pool(name="h1_ps", bufs=2, space="PSUM") as h1_ps, \
             tc.tile_pool(name="h2_ps", bufs=2, space="PSUM") as h2_ps, \
             tc.tile_pool(name="h1r_p", bufs=4) as h1r_p:

            state = {}

            def emit_w1(ui):
                s, mi, h = units[ui]
                nv = min(1024, (ll - KOFF[mi]) - 1024 * h)
                h1t = h1_ps.tile([H, 2, 512], dt.float32, name="h1", tag="h1ps")
                for ci in range(2):
                    n = min(512, nv - 512 * ci)
                    c0 = 1024 * h + 512 * ci
                    nc.tensor.matmul(h1t[:, ci, 0:n], wd1[:, mi],
                                     stacks[:, s, :, c0:c0 + n],
                                     start=True, stop=True, perf_mode=PM.DoubleRow)
                h1tf = h1t.rearrange("p a b -> p (a b)")
                h1r = h1r_p.tile([H, 1024], dt.float16, name="h1r", tag="h1r")
                e = E1[ui]
                if e == "A":
                    nc.scalar.activation(out=h1r[:, 0:nv], in_=h1tf[:, 0:nv],
                                         func=AF.Relu, bias=bw[:, mi:mi + 1])
                else:
                    veng(e).tensor_scalar(out=h1r[:, 0:nv], in0=h1tf[:, 0:nv],
                                          scalar1=bw[:, mi:mi + 1], scalar2=0.0,
                                          op0=ALU.add, op1=ALU.max)
                state[ui] = (h1r, nv)

            def emit_w2(ui):
                s, mi, h = units[ui]
                h1r, nv = state.pop(ui)
                h2t = h2_ps.tile([H, 2, 512], dt.float32, name="h2", tag="h2ps")
                for ci in range(2):
                    n = min(512, nv - 512 * ci)
                    nc.tensor.matmul(h2t[:, ci, 0:n], wd2[:, mi],
                                     h1r[:, 512 * ci: 512 * ci + n],
                                     start=True, stop=True)
                h2tf = h2t.rearrange("p a b -> p (a b)")
                col = 32 * mi + 2 * s + h
                e = E2[ui]
                if e == "A":
                    nc.scalar.activation(out=scr["A"][:, 0:nv], in_=h2tf[:, 0:nv],
                                         func=AF.Relu, bias=bw[:, 3 + mi:4 + mi],
                                         accum_out=acc[:, col:col + 1])
                else:
                    veng(e).scalar_tensor_tensor(out=scr[e][:, 0:nv], in0=h2tf[:, 0:nv],
                                                 scalar=bw[:, 3 + mi:4 + mi],
                                                 in1=zeros[:, 0:nv],
                                                 op0=ALU.add, op1=ALU.max,
                                                 accum_out=acc[:, col:col + 1])

            for ui in range(len(units) + LAG):
                if ui < len(units):
                    emit_w1(ui)
                if ui >= LAG:
                    emit_w2(ui - LAG)

        # ---------------- final reduction ----------------
        with tc.tile_pool(name="fin_ps", bufs=1, space="PSUM") as fin_ps:
            ep = fin_ps.tile([1, 3, 2 * bpc], dt.float32, name="ep")
            for mi in range(3):
                nc.tensor.matmul(ep[:, mi, :], bw[:, 6 + mi:7 + mi],
                                 acc[:, 32 * mi: 32 * mi + 2 * bpc], start=True, stop=True)
            esum = consts.tile([1, bpc], dt.float32, name="esum")
            nc.vector.tensor_reduce(
                out=esum, in_=ep.rearrange("o m (s h) -> o s m h", h=2), axis=AX.XY, op=ALU.add)
            eout = consts.tile([1, bpc], dt.float32, name="eout")
            nc.vector.tensor_scalar(out=eout, in0=esum, scalar1=bw[0:1, 9:10],
                                    scalar2=None, op0=ALU.add)
            nc.sync.dma_start(out=out_d.ap(), in_=eout)

    nc.finalize()
    return nc


_NC_CACHE = {}


def get_nc(bpc=BPC, ll=L):
    key = (bpc, ll)
    if key not in _NC_CACHE:
        _NC_CACHE[key] = build_nc(bpc, ll)
    return _NC_CACHE[key]


def _sel_matrices():
    SEL = np.zeros((48, 128), np.float16)
    # rotation: A1 col block 0:48 (A1[16c+i] = Ds[16((c+1)%3)+i]),
    #           A2 col block 64:112
    for c in range(3):
        for i in range(16):
            SEL[16 * ((c + 1) % 3) + i, 16 * c + i] = 1.0
            SEL[16 * ((c + 2) % 3) + i, 64 + 16 * c + i] = 1.0
            SEL[16 * c + i, 112 + i] = 1.0
    return SEL


def compute_comp(inputs, nsamp=8):
    """fp8 bias compensation: per-hidden-unit mean-shift corrections measured
    on a subsample of positions (device-quantized path vs fp32 reference)."""
    F16 = np.float16
    Rc = np.asarray(inputs["R"], np.float64)[:nsamp]
    seqc = np.asarray(inputs["seq"])[:nsamp]
    embf = np.asarray(inputs["emb"], np.float32)
    emb8 = embf.astype(FP8).astype(np.float32)
    d = Rc[:, 1:] - Rc[:, :-1]
    lens = np.linalg.norm(d, axis=-1)
    cos_t = -np.einsum('sld,sld->sl', d[:, :-1], d[:, 1:]) / (lens[:, :-1] * lens[:, 1:])
    n1 = np.cross(d[:, :-2], d[:, 1:-1])
    n2 = np.cross(d[:, 1:-1], d[:, 2:])
    x_ = np.einsum('sld,sld->sl', n1, n2)
    y_ = np.einsum('sld,sld->sl', d[:, :-2], n2) * lens[:, 1:-1]
    qn = np.sqrt(x_ ** 2 + y_ ** 2 + 1e-12)
    sin_p, cos_p = -y_ / qn, x_ / qn
    qf = lambda a: a.astype(F16).astype(FP8).astype(np.float32)
    e_ref, e_dev = embf[seqc], emb8[seqc]
    feats = {
        "fl": [lens - 3.8], "ft": [cos_t], "fp": [sin_p, cos_p],
    }
    eslices = {
        "fl": [slice(0, -1), slice(1, None)],
        "ft": [slice(0, -2), slice(1, -1), slice(2, None)],
        "fp": [slice(0, -3), slice(1, -2), slice(2, -1), slice(3, None)],
    }
    comp, db3 = {}, 0.0
    for mi, m in enumerate(MLPS):
        fs, sls = feats[m], eslices[m]
        nv = fs[0].shape[1]
        x_refm = np.concatenate([f[..., None] for f in fs]
                                + [e_ref[:, sl][:, :nv] for sl in sls], axis=-1)
        x_devm = np.concatenate([qf(f)[..., None] for f in fs]
                                + [e_dev[:, sl][:, :nv] for sl in sls], axis=-1)
        x_refm = x_refm.reshape(-1, x_refm.shape[-1])
        x_devm = x_devm.reshape(-1, x_devm.shape[-1])
        W1 = np.asarray(inputs[f"{m}_W1"], np.float32)
        W2 = np.asarray(inputs[f"{m}_W2"], np.float32)
        W3 = np.asarray(inputs[f"{m}_W3"], np.float32)
        W1q = W1.astype(FP8).astype(np.float32)
        W2q = W2.astype(np.float16).astype(np.float32)
        b1 = np.asarray(inputs[f"{m}_b1"], np.float32) + (3.8 * W1[0] if m == "fl" else 0.0)
        b2 = np.asarray(inputs[f"{m}_b2"], np.float32)
        h1_ref = x_refm @ W1 + b1
        h1_dev = x_devm @ W1q + b1
        db1 = h1_ref.mean(0) - h1_dev.mean(0)
        h1r_ref = np.maximum(h1_ref, 0)
        h1r_dev = np.maximum(h1_dev + db1, 0).astype(np.float16).astype(np.float32)
        h2_ref = h1r_ref @ W2 + b2
        h2_dev = h1r_dev @ W2q + b2
        db2 = h2_ref.mean(0) - h2_dev.mean(0)
        e3_ref = np.maximum(h2_ref, 0) @ W3
        e3_dev = np.maximum(h2_dev + db2, 0) @ W3
        db3 += float((e3_ref - e3_dev).mean()) * (L - KOFF[mi])
        comp[m] = (db1, db2)
    return comp, db3


def pack_weights(inputs):
    f32 = lambda k: np.asarray(inputs[k], np.float32)
    WD1 = np.zeros((35, 3, 2, H), np.float32)
    w = f32("fl_W1")                        # [33,H]: [len, e0, e1]
    WD1[0:16, 0, 0] = w[1:17]
    WD1[16:32, 0, 0] = w[17:33]
    WD1[32, 0, 0] = w[0]                    # len-3.8 feature row
    w = f32("ft_W1")                        # [49,H]: [cos_t, e0, e1, e2]
    WD1[0:16, 1, 0] = w[1:17]
    WD1[16:32, 1, 0] = w[17:33]
    WD1[0:16, 1, 1] = w[33:49]
    WD1[34, 1, 1] = -w[0]                   # stored -cos(theta)
    w = f32("fp_W1")                        # [66,H]: [sin, cos, e0..e3]
    WD1[0:16, 2, 0] = w[2:18]
    WD1[16:32, 2, 0] = w[18:34]
    WD1[0:16, 2, 1] = w[34:50]
    WD1[16:32, 2, 1] = w[50:66]
    WD1[32, 2, 1] = -w[0]                   # stored -sin(phi)
    WD1[33, 2, 1] = w[1]                    # cos(phi)

    WD2 = np.zeros((H, 3, H), np.float32)
    for mi, m in enumerate(MLPS):
        WD2[:, mi] = f32(f"{m}_W2")

    comp, db3 = compute_comp(inputs)
    BW = np.zeros((H, 10), np.float32)
    BW[:, 0] = f32("fl_b1") + 3.8 * f32("fl_W1")[0]
    BW[:, 1] = f32("ft_b1")
    BW[:, 2] = f32("fp_b1")
    for mi, m in enumerate(MLPS):
        BW[:, mi] += comp[m][0]
        BW[:, 3 + mi] = f32(f"{m}_b2") + comp[m][1]
        BW[:, 6 + mi] = f32(f"{m}_W3")[:, 0]
    BW[0, 9] = (float(f32("fl_b3").reshape(-1)[0]) * NL
                + float(f32("ft_b3").reshape(-1)[0]) * NT
                + float(f32("fp_b3").reshape(-1)[0]) * NP
                + db3)
    return WD1.astype(FP8), WD2.astype(np.float16), BW


def make_in_maps(inputs, bpc=BPC, ncores=NCORES):
    WD1, WD2, BW = pack_weights(inputs)
    emb8 = np.asarray(inputs["emb"], np.float32).astype(FP8)
    seq = np.asarray(inputs["seq"], np.int64)
    R = np.asarray(inputs["R"], np.float32)
    e_all = emb8[seq]                        # [B, L, E] fp8
    consts = dict(SEL=_sel_matrices(), WD1=WD1, WD2=WD2, BW=BW)
    in_maps = []
    for c in range(ncores):
        sl = slice(c * bpc, (c + 1) * bpc)
        Rt = np.ascontiguousarray(R[sl].transpose(2, 0, 1))        # [3, bpc, L]
        e = e_all[sl]                                              # [bpc, L, E]
        SE = np.zeros((bpc, 32, 2, L), FP8)
        for shift in range(4):
            kt, blk = divmod(shift, 2)
            SE[:, 16 * blk:16 * blk + 16, kt, :L - shift] = e[:, shift:, :].transpose(0, 2, 1)
        m = dict(consts)
        m["Rt"] = Rt
        m["SE"] = SE
        in_maps.append(m)
    return in_maps


def kernel(**inputs):
    nc = get_nc()
    in_maps = make_in_maps(inputs)
    res = bass_utils.run_bass_kernel_spmd(nc, in_maps, core_ids=list(range(NCORES)))
    return np.concatenate([res.results[c]["out"][0] for c in range(NCORES)]).astype(np.float32)


# revision 14
# speedup vs baseline: 1.2410x; 1.2410x over previous
"""Trainium2 Bass kernel for nn_LocalEnergy (protein local-energy GNN), v3.3.

kernel(**inputs) takes FULL unsharded inputs (B=128), shards B across 8
NeuronCores (16 samples/core, pure data parallel), runs one Bass kernel
SPMD, gathers per-core [16] energies into the full [128] output.

v3.3 design (vs v2 baseline at 177us):
 - all-fp16 matmuls: stack [68, L] per sample = 64 emb-shift rows + 4
   geometry feature rows (len-3.8, -sin, cos, -cos_t); b1/b2 biases applied
   inside the relu/accum passes (no ones row).
 - the two PSUM-evacuation passes per unit (h1 relu, h2 relu+accum) are
   split across ACT and DVE by a greedy load balancer (Pool cannot access
   PSUM on TRN2 and is slow, so it only gets SBUF-side geometry ops).
 - geometry: single act table (ln/exp/square/copy/relu), shifted views
   instead of DMA copies, rotations via one PE matmul family, per-sample
   scalars all at psum partition base 0.
"""

import sys
import types
import numpy as np
from contextlib import ExitStack


def ensure_axon_hooks():
    """The container's antenv is a stub without axon_hooks; inject it so
    run_bass_kernel_spmd(trace=True) can NTFF-profile."""
    if "antenv.axon_hooks" in sys.modules:
        return
    import antenv

    hooks = types.ModuleType("antenv.axon_hooks")
    hooks._h = None

    def set_axon_ntff_profile_hook(h):
        hooks._h = h

    def get_axon_ntff_profile_hook():
        return hooks._h

    hooks.set_axon_ntff_profile_hook = set_axon_ntff_profile_hook
    hooks.get_axon_ntff_profile_hook = get_axon_ntff_profile_hook
    sys.modules["antenv.axon_hooks"] = hooks
    antenv.axon_hooks = hooks
    try:
        from trn_agent_boot.trn_boot import _ntff_profile_via_ctypes

        hook = _ntff_profile_via_ctypes("/opt/axon/libaxon_pjrt.so")
        if hook is not None:
            set_axon_ntff_profile_hook(hook)
    except Exception:
        pass


ensure_axon_hooks()

import concourse.bass as bass  # noqa: E402
import concourse.tile as tile  # noqa: E402
from concourse import mybir, bacc, bass_utils  # noqa: E402

dt = mybir.dt
AF = mybir.ActivationFunctionType
ALU = mybir.AluOpType
AX = mybir.AxisListType

NCORES = 8
B, L, NAA, E, H = 128, 2048, 20, 16, 128
BPC = B // NCORES
MLPS = ("fl", "ft", "fp")
KOFF = (1, 2, 3)                  # valid cols per sample = L - KOFF[m]
SINV = 1.0 / 16.0                 # bond-vector scaling to stay in fp16 range
NL, NT, NP = L - 1, L - 2, L - 3
KS = 68                           # stack rows: 64 emb-shift + 4 features

# per-unit engine cost estimates (us) used by the greedy scheduler:
# (relu unit, relu+accum unit); preload = geometry work per engine.
# Only ACT and DVE can access PSUM (Pool/gpsimd is SBUF-only on TRN2).
ECOST = {"A": (1.07, 1.35), "D": (1.26, 1.31)}
EPRELOAD = {"A": 13.0, "D": 17.0}
LAG = 3                           # units between W1 emission and W2 emission


def plan_engines():
    units = [(s, mi, h) for s in range(BPC) for mi in range(3) for h in range(2)]
    load = dict(EPRELOAD)
    e1, e2 = [], []
    for _ in units:
        e = min("AD", key=lambda k: load[k] + ECOST[k][0])
        e1.append(e)
        load[e] += ECOST[e][0]
        e = min("AD", key=lambda k: load[k] + ECOST[k][1])
        e2.append(e)
        load[e] += ECOST[e][1]
    return units, e1, e2


def build_nc(bpc=BPC, ll=L):
    nc = bacc.Bacc("TRN2", target_bir_lowering=False, debug=False)

    Rt_d = nc.dram_tensor("Rt", (3, bpc, ll), dt.float32, kind="ExternalInput")
    SE_d = nc.dram_tensor("SE", (bpc, 64, ll), dt.float16, kind="ExternalInput")
    SEL_d = nc.dram_tensor("SEL", (48, 128), dt.float16, kind="ExternalInput")
    WD1_d = nc.dram_tensor("WD1", (KS, 3, H), dt.float16, kind="ExternalInput")
    WD2_d = nc.dram_tensor("WD2", (H, 3, H), dt.float16, kind="ExternalInput")
    BW_d = nc.dram_tensor("BW", (H, 10), dt.float32, kind="ExternalInput")
    out_d = nc.dram_tensor("out", (1, bpc), dt.float32, kind="ExternalOutput")

    nl, nt, np_ = ll - 1, ll - 2, ll - 3
    units, E1, E2 = plan_engines()

    def veng(e):
        return {"D": nc.vector, "P": nc.gpsimd}[e]

    with tile.TileContext(nc) as tc, ExitStack() as ctx:
        consts = ctx.enter_context(tc.tile_pool(name="consts", bufs=1))

        rt = consts.tile([48, ll], dt.float32, name="rt")
        for c in range(3):
            nc.sync.dma_start(out=rt[16 * c: 16 * c + bpc, :], in_=Rt_d.ap()[c])
        sel = consts.tile([48, 128], dt.float16, name="sel")
        nc.sync.dma_start(out=sel, in_=SEL_d.ap())
        srot, s48 = sel[:, 0:112], sel[:, 112:128]
        wd1 = consts.tile([KS, 3, H], dt.float16, name="wd1")
        nc.sync.dma_start(out=wd1, in_=WD1_d.ap())
        wd2 = consts.tile([H, 3, H], dt.float16, name="wd2")
        nc.sync.dma_start(out=wd2, in_=WD2_d.ap())
        bw = consts.tile([H, 10], dt.float32, name="bw")
        nc.sync.dma_start(out=bw, in_=BW_d.ap())

        # persistent per-sample stacks [68, s, L] fp16
        stacks = consts.tile([KS, bpc, ll], dt.float16, name="stacks")
        for s in range(bpc):
            nc.sync.dma_start(out=stacks[0:64, s], in_=SE_d.ap()[s])

        # feature tile [sample, j, L]; j: 0=len-3.8, 1=-sin, 2=cos, 3=-cos_t
        F = consts.tile([16, 4, ll], dt.float16, name="F")
        nc.vector.memset(F[:, :, ll - 8: ll], 0.0)
        eps_t = consts.tile([16, 1], dt.float32, name="eps_t")
        nc.vector.memset(eps_t, 1e-6)
        zeros = consts.tile([H, 1024], dt.float16, name="zeros")
        nc.vector.memset(zeros, 0.0)
        scr = {}
        for e in "AD":
            scr[e] = consts.tile([H, 1024], dt.float16, name=f"scr{e}")
        acc = consts.tile([H, 96], dt.float32, name="acc")

        # ---------------- Phase 1: geometry ----------------
        with tc.tile_pool(name="geo", bufs=1) as geo, \
             tc.tile_pool(name="geo_ps", bufs=2, space="PSUM") as geo_ps:

            D = geo.tile([48, nl], dt.float16, name="D")
            nc.vector.tensor_tensor(out=D, in0=rt[:, 1:ll], in1=rt[:, 0:nl], op=ALU.subtract)
            Ds = geo.tile([48, nl], dt.float16, name="Ds")
            nc.vector.tensor_scalar(out=Ds, in0=D, scalar1=SINV, scalar2=None, op0=ALU.mult)
            DSQ = geo.tile([48, nl], dt.float16, name="DSQ")
            nc.vector.tensor_tensor(out=DSQ, in0=D, in1=D, op=ALU.mult)
            DD = geo.tile([48, nt], dt.float16, name="DD")
            nc.vector.tensor_tensor(out=DD, in0=D[:, 0:nt], in1=D[:, 1:nl], op=ALU.mult)

            def chunked_mm(dst_flat, lhsT, src, count):
                for c0 in range(0, count, 512):
                    n = min(512, count - c0)
                    nc.tensor.matmul(dst_flat[:, c0:c0 + n], lhsT, src[:, c0:c0 + n],
                                     start=True, stop=True)

            # all per-sample scalars go to psum partition base 0 (one tile each)
            ld1 = geo_ps.tile([128, 4, 512], dt.float32, name="ld1", tag="gps")
            ld1f = ld1.rearrange("p a b -> p (a b)")
            chunked_mm(ld1f[0:16, :], s48, DSQ, nl)               # lsq
            lnl = geo.tile([16, nl], dt.float16, name="lnl")
            nc.scalar.activation(out=lnl, in_=ld1f[0:16, 0:nl], func=AF.Ln)
            rlen = geo.tile([16, nl], dt.float16, name="rlen")
            nc.scalar.activation(out=rlen, in_=lnl, func=AF.Exp, scale=-0.5)
            lent = geo.tile([16, nl], dt.float16, name="lent")
            nc.scalar.activation(out=lent, in_=lnl, func=AF.Exp, scale=0.5)
            nc.gpsimd.tensor_scalar(out=F[:, 0, 0:nl], in0=lent, scalar1=3.8,
                                    scalar2=None, op0=ALU.subtract)

            ld2 = geo_ps.tile([128, 4, 512], dt.float32, name="ld2", tag="gps")
            ld2f = ld2.rearrange("p a b -> p (a b)")
            chunked_mm(ld2f[0:16, :], s48, DD, nt)                # dot
            tt1 = geo.tile([16, nt], dt.float16, name="tt1")
            nc.vector.tensor_tensor(out=tt1, in0=ld2f[0:16, 0:nt], in1=rlen[:, 0:nt], op=ALU.mult)
            # -cos(theta) = +dot*rlen*rlen1; sign folded into W1
            nc.gpsimd.tensor_tensor(out=F[:, 3, 0:nt], in0=tt1, in1=rlen[:, 1:nl], op=ALU.mult)

            # rotations A1@0:48, A2@64:112 via one matmul family
            pa = geo_ps.tile([128, 4, 512], dt.float32, name="pa", tag="gps")
            paf = pa.rearrange("p a b -> p (a b)")
            chunked_mm(paf[0:112, :], srot, Ds, nl)
            a1sb = geo.tile([48, nl], dt.float16, name="a1sb")
            nc.scalar.activation(out=a1sb, in_=paf[0:48, 0:nl], func=AF.Copy)
            a2sb = geo.tile([48, nl], dt.float16, name="a2sb")
            nc.vector.tensor_scalar(out=a2sb, in0=paf[64:112, 0:nl], scalar1=0.0,
                                    scalar2=None, op0=ALU.add)
            t_a = geo.tile([48, nt], dt.float16, name="t_a")
            nc.vector.tensor_tensor(out=t_a, in0=a1sb[:, 0:nt], in1=a2sb[:, 1:nl], op=ALU.mult)
            t_b = geo.tile([48, nt], dt.float16, name="t_b")
            nc.vector.tensor_tensor(out=t_b, in0=a2sb[:, 0:nt], in1=a1sb[:, 1:nl], op=ALU.mult)
            Cs = geo.tile([48, nt], dt.float16, name="Cs")
            nc.vector.tensor_tensor(out=Cs, in0=t_a, in1=t_b, op=ALU.subtract)
            XRp = geo.tile([48, np_], dt.float16, name="XRp")
            nc.vector.tensor_tensor(out=XRp, in0=Cs[:, 0:np_], in1=Cs[:, 1:nt], op=ALU.mult)
            YRp = geo.tile([48, np_], dt.float16, name="YRp")
            nc.gpsimd.tensor_tensor(out=YRp, in0=Ds[:, 0:np_], in1=Cs[:, 1:nt], op=ALU.mult)

            xyx = geo_ps.tile([128, 4, 512], dt.float32, name="xyx", tag="gps")
            xyxf = xyx.rearrange("p a b -> p (a b)")
            chunked_mm(xyxf[0:16, :], s48, XRp, np_)              # x = n1.n2
            xx = geo.tile([16, np_], dt.float16, name="xx")
            nc.scalar.activation(out=xx, in_=xyxf[0:16, 0:np_], func=AF.Square)

            xyy = geo_ps.tile([128, 4, 512], dt.float32, name="xyy", tag="gps")
            xyyf = xyy.rearrange("p a b -> p (a b)")
            chunked_mm(xyyf[0:16, :], s48, YRp, np_)              # y-raw
            y_t = geo.tile([16, np_], dt.float16, name="y_t")
            nc.vector.scalar_tensor_tensor(out=y_t, in0=xyyf[0:16, 0:np_], scalar=SINV,
                                           in1=lent[:, 1:1 + np_], op0=ALU.mult, op1=ALU.mult)
            yy = geo.tile([16, np_], dt.float16, name="yy")
            nc.gpsimd.tensor_tensor(out=yy, in0=y_t, in1=y_t, op=ALU.mult)
            q = geo.tile([16, np_], dt.float16, name="q")
            nc.gpsimd.tensor_tensor(out=q, in0=xx, in1=yy, op=ALU.add)
            lnq = geo.tile([16, np_], dt.float16, name="lnq")
            nc.scalar.activation(out=lnq, in_=q, func=AF.Ln, bias=eps_t)
            r2 = geo.tile([16, np_], dt.float16, name="r2")
            nc.scalar.activation(out=r2, in_=lnq, func=AF.Exp, scale=-0.5)
            # -sin(phi) convention: stored y*r2 matches v2; W1 row = -w
            nc.gpsimd.tensor_tensor(out=F[:, 1, 0:np_], in0=y_t, in1=r2, op=ALU.mult)
            nc.vector.tensor_tensor(out=F[:, 2, 0:np_], in0=xyxf[0:16, 0:np_], in1=r2, op=ALU.mult)

        # per-sample feature DMA into the stacks (rows 64:68)
        for s in range(bpc):
            nc.sync.dma_start(out=stacks[64:68, s, :], in_=F[s:s + 1, :, :])

        # ---------------- Phase 2: MLP units ----------------
        with tc.tile_pool(name="h1_ps", bufs=2, space="PSUM") as h1_ps, \
             tc.tile_pool(name="h2_ps", bufs=2, space="PSUM") as h2_ps, \
             tc.tile_pool(name="h1r_p", bufs=4) as h1r_p:

            state = {}

            def emit_w1(ui):
                s, mi, h = units[ui]
                nv = min(1024, (ll - KOFF[mi]) - 1024 * h)
                h1t = h1_ps.tile([H, 2, 512], dt.float32, name="h1", tag="h1ps")
                for ci in range(2):
                    n = min(512, nv - 512 * ci)
                    c0 = 1024 * h + 512 * ci
                    nc.tensor.matmul(h1t[:, ci, 0:n], wd1[:, mi],
                                     stacks[:, s, c0:c0 + n], start=True, stop=True)
                h1tf = h1t.rearrange("p a b -> p (a b)")
                h1r = h1r_p.tile([H, 1024], dt.float16, name="h1r", tag="h1r")
                e = E1[ui]
                if e == "A":
                    nc.scalar.activation(out=h1r[:, 0:nv], in_=h1tf[:, 0:nv],
                                         func=AF.Relu, bias=bw[:, mi:mi + 1])
                else:
                    veng(e).tensor_scalar(out=h1r[:, 0:nv], in0=h1tf[:, 0:nv],
                                          scalar1=bw[:, mi:mi + 1], scalar2=0.0,
                                          op0=ALU.add, op1=ALU.max)
                state[ui] = (h1r, nv)

            def emit_w2(ui):
                s, mi, h = units[ui]
                h1r, nv = state.pop(ui)
                h2t = h2_ps.tile([H, 2, 512], dt.float32, name="h2", tag="h2ps")
                for ci in range(2):
                    n = min(512, nv - 512 * ci)
                    nc.tensor.matmul(h2t[:, ci, 0:n], wd2[:, mi],
                                     h1r[:, 512 * ci: 512 * ci + n],
                                     start=True, stop=True)
                h2tf = h2t.rearrange("p a b -> p (a b)")
                col = 32 * mi + 2 * s + h
                e = E2[ui]
                if e == "A":
                    nc.scalar.activation(out=scr["A"][:, 0:nv], in_=h2tf[:, 0:nv],
                                         func=AF.Relu, bias=bw[:, 3 + mi:4 + mi],
                                         accum_out=acc[:, col:col + 1])
                else:
                    veng(e).scalar_tensor_tensor(out=scr[e][:, 0:nv], in0=h2tf[:, 0:nv],
                                                 scalar=bw[:, 3 + mi:4 + mi],
                                                 in1=zeros[:, 0:nv],
                                                 op0=ALU.add, op1=ALU.max,
                                                 accum_out=acc[:, col:col + 1])

            for ui in range(len(units) + LAG):
                if ui < len(units):
                    emit_w1(ui)
                if ui >= LAG:
                    emit_w2(ui - LAG)

        # ---------------- final reduction ----------------
        with tc.tile_pool(name="fin_ps", bufs=1, space="PSUM") as fin_ps:
            ep = fin_ps.tile([1, 3, 2 * bpc], dt.float32, name="ep")
            for mi in range(3):
                nc.tensor.matmul(ep[:, mi, :], bw[:, 6 + mi:7 + mi],
                                 acc[:, 32 * mi: 32 * mi + 2 * bpc], start=True, stop=True)
            esum = consts.tile([1, bpc], dt.float32, name="esum")
            nc.vector.tensor_reduce(
                out=esum, in_=ep.rearrange("o m (s h) -> o s m h", h=2), axis=AX.XY, op=ALU.add)
            eout = consts.tile([1, bpc], dt.float32, name="eout")
            nc.vector.tensor_scalar(out=eout, in0=esum, scalar1=bw[0:1, 9:10],
                                    scalar2=None, op0=ALU.add)
            nc.sync.dma_start(out=out_d.ap(), in_=eout)

    nc.finalize()
    return nc


_NC_CACHE = {}


def get_nc(bpc=BPC, ll=L):
    key = (bpc, ll)
    if key not in _NC_CACHE:
        _NC_CACHE[key] = build_nc(bpc, ll)
    return _NC_CACHE[key]


def _sel_matrices():
    SEL = np.zeros((48, 128), np.float16)
    # rotation: A1 col block 0:48 (A1[16c+i] = Ds[16((c+1)%3)+i]),
    #           A2 col block 64:112; S48 selection at cols 112:128
    for c in range(3):
        for i in range(16):
            SEL[16 * ((c + 1) % 3) + i, 16 * c + i] = 1.0
            SEL[16 * ((c + 2) % 3) + i, 64 + 16 * c + i] = 1.0
            SEL[16 * c + i, 112 + i] = 1.0
    return SEL


def pack_weights(inputs):
    f32 = lambda k: np.asarray(inputs[k], np.float32)
    WD1 = np.zeros((KS, 3, H), np.float32)
    w = f32("fl_W1")                        # [33,H]: [len, e0, e1]
    WD1[0:32, 0] = w[1:33]
    WD1[64, 0] = w[0]                       # len-3.8 feature row
    w = f32("ft_W1")                        # [49,H]: [cos_t, e0, e1, e2]
    WD1[0:48, 1] = w[1:49]
    WD1[67, 1] = -w[0]                      # stored -cos(theta)
    w = f32("fp_W1")                        # [66,H]: [sin, cos, e0..e3]
    WD1[0:64, 2] = w[2:66]
    WD1[65, 2] = -w[0]                      # stored -sin(phi)
    WD1[66, 2] = w[1]                       # cos(phi)

    WD2 = np.zeros((H, 3, H), np.float32)
    for mi, m in enumerate(MLPS):
        WD2[:, mi] = f32(f"{m}_W2")

    BW = np.zeros((H, 10), np.float32)
    BW[:, 0] = f32("fl_b1") + 3.8 * f32("fl_W1")[0]
    BW[:, 1] = f32("ft_b1")
    BW[:, 2] = f32("fp_b1")
    for mi, m in enumerate(MLPS):
        BW[:, 3 + mi] = f32(f"{m}_b2")
        BW[:, 6 + mi] = f32(f"{m}_W3")[:, 0]
    BW[0, 9] = (float(f32("fl_b3").reshape(-1)[0]) * NL
                + float(f32("ft_b3").reshape(-1)[0]) * NT
                + float(f32("fp_b3").reshape(-1)[0]) * NP)
    return WD1.astype(np.float16), WD2.astype(np.float16), BW


def make_in_maps(inputs, bpc=BPC, ncores=NCORES):
    WD1, WD2, BW = pack_weights(inputs)
    emb16 = np.asarray(inputs["emb"], np.float32).astype(np.float16)
    seq = np.asarray(inputs["seq"], np.int64)
    R = np.asarray(inputs["R"], np.float32)
    e_all = emb16[seq]                       # [B, L, E]
    consts = dict(SEL=_sel_matrices(), WD1=WD1, WD2=WD2, BW=BW)
    in_maps = []
    for c in range(ncores):
        sl = slice(c * bpc, (c + 1) * bpc)
        Rt = np.ascontiguousarray(R[sl].transpose(2, 0, 1))        # [3, bpc, L]
        e = e_all[sl]                                              # [bpc, L, E]
        SE = np.zeros((bpc, 64, L), np.float16)
        for shift in range(4):
            SE[:, 16 * shift:16 * shift + 16, :L - shift] = e[:, shift:, :].transpose(0, 2, 1)
        m = dict(consts)
        m["Rt"] = Rt
        m["SE"] = SE
        in_maps.append(m)
    return in_maps


def kernel(**inputs):
    nc = get_nc()
    in_maps = make_in_maps(inputs)
    res = bass_utils.run_bass_kernel_spmd(nc, in_maps, core_ids=list(range(NCORES)))
    return np.concatenate([res.results[c]["out"][0] for c in range(NCORES)]).astype(np.float32)


# revision 16
# speedup vs baseline: 1.4957x; 1.2052x over previous
"""Trainium2 Bass kernel for nn_LocalEnergy (protein local-energy GNN), v3.3.

kernel(**inputs) takes FULL unsharded inputs (B=128), shards B across 8
NeuronCores (16 samples/core, pure data parallel), runs one Bass kernel
SPMD, gathers per-core [16] energies into the full [128] output.

v3.3 design (vs v2 baseline at 177us):
 - all-fp16 matmuls: stack [68, L] per sample = 64 emb-shift rows + 4
   geometry feature rows (len-3.8, -sin, cos, -cos_t); b1/b2 biases applied
   inside the relu/accum passes (no ones row).
 - the two PSUM-evacuation passes per unit (h1 relu, h2 relu+accum) are
   split across ACT and DVE by a greedy load balancer (Pool cannot access
   PSUM on TRN2 and is slow, so it only gets SBUF-side geometry ops).
 - geometry: single act table (ln/exp/square/copy/relu), shifted views
   instead of DMA copies, rotations via one PE matmul family, per-sample
   scalars all at psum partition base 0.
"""

import sys
import types
import numpy as np
from contextlib import ExitStack


def ensure_axon_hooks():
    """The container's antenv is a stub without axon_hooks; inject it so
    run_bass_kernel_spmd(trace=True) can NTFF-profile."""
    if "antenv.axon_hooks" in sys.modules:
        return
    import antenv

    hooks = types.ModuleType("antenv.axon_hooks")
    hooks._h = None

    def set_axon_ntff_profile_hook(h):
        hooks._h = h

    def get_axon_ntff_profile_hook():
        return hooks._h

    hooks.set_axon_ntff_profile_hook = set_axon_ntff_profile_hook
    hooks.get_axon_ntff_profile_hook = get_axon_ntff_profile_hook
    sys.modules["antenv.axon_hooks"] = hooks
    antenv.axon_hooks = hooks
    try:
        from trn_agent_boot.trn_boot import _ntff_profile_via_ctypes

        hook = _ntff_profile_via_ctypes("/opt/axon/libaxon_pjrt.so")
        if hook is not None:
            set_axon_ntff_profile_hook(hook)
    except Exception:
        pass


ensure_axon_hooks()

import concourse.bass as bass  # noqa: E402
import concourse.tile as tile  # noqa: E402
from concourse import mybir, bacc, bass_utils  # noqa: E402

dt = mybir.dt
AF = mybir.ActivationFunctionType
ALU = mybir.AluOpType
AX = mybir.AxisListType

NCORES = 8
B, L, NAA, E, H = 128, 2048, 20, 16, 128
BPC = B // NCORES
MLPS = ("fl", "ft", "fp")
KOFF = (1, 2, 3)                  # valid cols per sample = L - KOFF[m]
SINV = 1.0 / 16.0                 # bond-vector scaling to stay in fp16 range
NL, NT, NP = L - 1, L - 2, L - 3
KS = 68                           # stack rows: 64 emb-shift + 4 features

# per-unit engine cost estimates (us) used by the greedy scheduler:
# (relu unit, relu+accum unit); preload = geometry work per engine.
# Only ACT and DVE can access PSUM (Pool/gpsimd is SBUF-only on TRN2).
ECOST = {"A": (1.07, 1.35), "D": (1.26, 1.31)}
EPRELOAD = {"A": 13.0, "D": 17.0}
LAG = 3                           # units between W1 emission and W2 emission


def plan_engines():
    units = [(s, mi, h) for s in range(BPC) for mi in range(3) for h in range(2)]
    load = dict(EPRELOAD)
    e1, e2 = [], []
    for _ in units:
        e = min("AD", key=lambda k: load[k] + ECOST[k][0])
        e1.append(e)
        load[e] += ECOST[e][0]
        e = min("AD", key=lambda k: load[k] + ECOST[k][1])
        e2.append(e)
        load[e] += ECOST[e][1]
    return units, e1, e2


def build_nc(bpc=BPC, ll=L):
    nc = bacc.Bacc("TRN2", target_bir_lowering=False, debug=False)

    Rt_d = nc.dram_tensor("Rt", (96, 1028), dt.float32, kind="ExternalInput")
    SE_d = nc.dram_tensor("SE", (bpc, 64, ll), dt.float16, kind="ExternalInput")
    SEL_d = nc.dram_tensor("SEL", (96, 240), dt.float16, kind="ExternalInput")
    WD1_d = nc.dram_tensor("WD1", (KS, 3, H), dt.float16, kind="ExternalInput")
    WD2_d = nc.dram_tensor("WD2", (H, 3, H), dt.float16, kind="ExternalInput")
    BW_d = nc.dram_tensor("BW", (H, 10), dt.float32, kind="ExternalInput")
    out_d = nc.dram_tensor("out", (1, bpc), dt.float32, kind="ExternalOutput")

    nl, nt, np_ = ll - 1, ll - 2, ll - 3
    units, E1, E2 = plan_engines()

    def veng(e):
        return {"D": nc.vector, "P": nc.gpsimd}[e]

    with tile.TileContext(nc) as tc, ExitStack() as ctx:
        consts = ctx.enter_context(tc.tile_pool(name="consts", bufs=1))

        # rt2: two column-halves packed on partitions (p = 48*half + 16c + s);
        # half0 = cols 0:1028, half1 = cols 1020:2048 (8-col overlap)
        rt2 = consts.tile([96, 1028], dt.float32, name="rt2")
        nc.sync.dma_start(out=rt2, in_=Rt_d.ap())
        sel = consts.tile([96, 240], dt.float16, name="sel")
        nc.sync.dma_start(out=sel, in_=SEL_d.ap())
        s96, srot1, srot2 = sel[:, 0:48], sel[:, 48:144], sel[:, 144:240]
        wd1 = consts.tile([KS, 3, H], dt.float16, name="wd1")
        nc.sync.dma_start(out=wd1, in_=WD1_d.ap())
        wd2 = consts.tile([H, 3, H], dt.float16, name="wd2")
        nc.sync.dma_start(out=wd2, in_=WD2_d.ap())
        bw = consts.tile([H, 10], dt.float32, name="bw")
        nc.sync.dma_start(out=bw, in_=BW_d.ap())

        # persistent per-sample stacks [68, s, L] fp16
        stacks = consts.tile([KS, bpc, ll], dt.float16, name="stacks")
        for s in range(bpc):
            nc.sync.dma_start(out=stacks[0:64, s], in_=SE_d.ap()[s])

        # feature tile [sample, j, L]; j: 0=len-3.8, 1=-sin, 2=cos, 3=-cos_t
        F = consts.tile([16, 4, ll], dt.float16, name="F")
        nc.vector.memset(F[:, :, ll - 8: ll], 0.0)
        eps_t = consts.tile([48, 1], dt.float32, name="eps_t")
        nc.vector.memset(eps_t, 1e-6)
        zeros = consts.tile([H, 1024], dt.float16, name="zeros")
        nc.vector.memset(zeros, 0.0)
        scr = {}
        for e in "AD":
            scr[e] = consts.tile([H, 1024], dt.float16, name=f"scr{e}")
        acc = consts.tile([H, 96], dt.float32, name="acc")

        # ---------------- Phase 1: geometry ----------------
        # All tiles hold two column-halves on partitions: rows 0:48 = half0
        # (global cols 0:1028), rows 48:96 = half1 (global 1020:2048).
        # Per-sample scalars land at partition rows 0:16 (half0) / 32:48
        # (half1). +-1 column shifts are materialized via aligned DMA copies
        # (odd fp16 column offsets halve DVE throughput).
        HB = 1024 - 4                      # half1 global base column
        HC = 1028                          # half width
        with tc.tile_pool(name="geo", bufs=1) as geo, \
             tc.tile_pool(name="geo_ps", bufs=2, space="PSUM") as geo_ps:

            nd = HC - 1                    # d columns per half
            ndd = HC - 2
            npp = HC - 3

            D2 = geo.tile([96, nd], dt.float16, name="D2")
            nc.vector.tensor_tensor(out=D2, in0=rt2[:, 1:HC], in1=rt2[:, 0:nd], op=ALU.subtract)
            D12 = geo.tile([96, ndd], dt.float16, name="D12")
            nc.vector.tensor_tensor(out=D12, in0=rt2[:, 2:HC], in1=rt2[:, 1:nd], op=ALU.subtract)
            Ds2 = geo.tile([96, nd], dt.float16, name="Ds2")
            nc.vector.tensor_scalar(out=Ds2, in0=D2, scalar1=SINV, scalar2=None, op0=ALU.mult)
            D1s2 = geo.tile([96, ndd], dt.float16, name="D1s2")
            nc.vector.tensor_scalar(out=D1s2, in0=D12, scalar1=SINV, scalar2=None, op0=ALU.mult)
            DSQ = geo.tile([96, nd], dt.float16, name="DSQ")
            nc.vector.tensor_tensor(out=DSQ, in0=D2, in1=D2, op=ALU.mult)
            DD = geo.tile([96, ndd], dt.float16, name="DD")
            nc.vector.tensor_tensor(out=DD, in0=D2[:, 0:ndd], in1=D12, op=ALU.mult)

            def chunked_mm(dst_flat, lhsT, src, count):
                for c0 in range(0, count, 512):
                    n = min(512, count - c0)
                    nc.tensor.matmul(dst_flat[:, c0:c0 + n], lhsT, src[:, c0:c0 + n],
                                     start=True, stop=True)

            # lsq / dot at rows 0:16 (half0), 32:48 (half1)
            ld1 = geo_ps.tile([128, 4, 512], dt.float32, name="ld1", tag="gps")
            ld1f = ld1.rearrange("p a b -> p (a b)")
            chunked_mm(ld1f[0:48, :], s96, DSQ, nd)
            lnl = geo.tile([48, nd], dt.float16, name="lnl")
            nc.scalar.activation(out=lnl, in_=ld1f[0:48, 0:nd], func=AF.Ln)
            rlen = geo.tile([48, nd], dt.float16, name="rlen")
            nc.scalar.activation(out=rlen, in_=lnl, func=AF.Exp, scale=-0.5)
            lent = geo.tile([48, nd], dt.float16, name="lent")
            nc.scalar.activation(out=lent, in_=lnl, func=AF.Exp, scale=0.5)
            rlen1 = geo.tile([48, ndd], dt.float16, name="rlen1")
            nc.sync.dma_start(out=rlen1, in_=rlen[:, 1:nd])
            lent1 = geo.tile([48, npp], dt.float16, name="lent1")
            nc.sync.dma_start(out=lent1, in_=lent[:, 1:1 + npp])
            # len feature
            nc.vector.tensor_scalar(out=F[:, 0, 0:1024], in0=lent[0:16, 0:1024],
                                    scalar1=3.8, scalar2=None, op0=ALU.subtract)
            nc.vector.tensor_scalar(out=F[:, 0, 1024:2047], in0=lent[32:48, 4:1027],
                                    scalar1=3.8, scalar2=None, op0=ALU.subtract)

            ld2 = geo_ps.tile([128, 4, 512], dt.float32, name="ld2", tag="gps")
            ld2f = ld2.rearrange("p a b -> p (a b)")
            chunked_mm(ld2f[0:48, :], s96, DD, ndd)
            tt1 = geo.tile([48, ndd], dt.float16, name="tt1")
            nc.vector.tensor_tensor(out=tt1, in0=ld2f[0:48, 0:ndd], in1=rlen[:, 0:ndd], op=ALU.mult)
            # -cos(theta) feature = +dot*rlen*rlen1; sign folded into W1
            nc.vector.tensor_tensor(out=F[:, 3, 0:1024], in0=tt1[0:16, 0:1024],
                                    in1=rlen1[0:16, 0:1024], op=ALU.mult)
            nc.vector.tensor_tensor(out=F[:, 3, 1024:2046], in0=tt1[32:48, 4:1026],
                                    in1=rlen1[32:48, 4:1026], op=ALU.mult)

            # rotations: A = rot(Ds2), B = rot(D1s2); B copied to SBUF so the
            # cross products read one PSUM + one SBUF operand
            pa1 = geo_ps.tile([128, 4, 512], dt.float32, name="pa1", tag="gps")
            pa1f = pa1.rearrange("p a b -> p (a b)")
            chunked_mm(pa1f[0:96, :], srot1, Ds2, ndd)
            pb2 = geo_ps.tile([128, 4, 512], dt.float32, name="pb2", tag="gps")
            pb2f = pb2.rearrange("p a b -> p (a b)")
            chunked_mm(pb2f[0:96, :], srot2, D1s2, ndd)
            b2sb = geo.tile([96, ndd], dt.float16, name="b2sb")
            nc.scalar.activation(out=b2sb, in_=pb2f[0:96, 0:ndd], func=AF.Copy)
            t_a = geo.tile([96, ndd], dt.float16, name="t_a")
            nc.vector.tensor_tensor(out=t_a, in0=pa1f[0:96, 0:ndd], in1=b2sb, op=ALU.mult)

            pa2 = geo_ps.tile([128, 4, 512], dt.float32, name="pa2", tag="gps")
            pa2f = pa2.rearrange("p a b -> p (a b)")
            chunked_mm(pa2f[0:96, :], srot2, Ds2, ndd)
            pb1 = geo_ps.tile([128, 4, 512], dt.float32, name="pb1", tag="gps")
            pb1f = pb1.rearrange("p a b -> p (a b)")
            chunked_mm(pb1f[0:96, :], srot1, D1s2, ndd)
            b1sb = geo.tile([96, ndd], dt.float16, name="b1sb")
            nc.scalar.activation(out=b1sb, in_=pb1f[0:96, 0:ndd], func=AF.Copy)
            t_b = geo.tile([96, ndd], dt.float16, name="t_b")
            nc.vector.tensor_tensor(out=t_b, in0=pa2f[0:96, 0:ndd], in1=b1sb, op=ALU.mult)

            Cs = geo.tile([96, ndd], dt.float16, name="Cs")
            nc.vector.tensor_tensor(out=Cs, in0=t_a, in1=t_b, op=ALU.subtract)
            Cs1 = geo.tile([96, npp], dt.float16, name="Cs1")
            nc.sync.dma_start(out=Cs1, in_=Cs[:, 1:1 + npp])
            XRp = geo.tile([96, npp], dt.float16, name="XRp")
            nc.vector.tensor_tensor(out=XRp, in0=Cs[:, 0:npp], in1=Cs1, op=ALU.mult)
            YRp = geo.tile([96, npp], dt.float16, name="YRp")
            nc.vector.tensor_tensor(out=YRp, in0=Ds2[:, 0:npp], in1=Cs1, op=ALU.mult)

            xyx = geo_ps.tile([128, 4, 512], dt.float32, name="xyx", tag="gps")
            xyxf = xyx.rearrange("p a b -> p (a b)")
            chunked_mm(xyxf[0:48, :], s96, XRp, npp)              # x = n1.n2
            xx = geo.tile([48, npp], dt.float16, name="xx")
            nc.scalar.activation(out=xx, in_=xyxf[0:48, 0:npp], func=AF.Square)

            xyy = geo_ps.tile([128, 4, 512], dt.float32, name="xyy", tag="gps")
            xyyf = xyy.rearrange("p a b -> p (a b)")
            chunked_mm(xyyf[0:48, :], s96, YRp, npp)              # y-raw
            y_t = geo.tile([48, npp], dt.float16, name="y_t")
            nc.vector.scalar_tensor_tensor(out=y_t, in0=xyyf[0:48, 0:npp], scalar=SINV,
                                           in1=lent1, op0=ALU.mult, op1=ALU.mult)
            yy = geo.tile([48, npp], dt.float16, name="yy")
            nc.vector.tensor_tensor(out=yy, in0=y_t, in1=y_t, op=ALU.mult)
            q = geo.tile([48, npp], dt.float16, name="q")
            nc.vector.tensor_tensor(out=q, in0=xx, in1=yy, op=ALU.add)
            lnq = geo.tile([48, npp], dt.float16, name="lnq")
            nc.scalar.activation(out=lnq, in_=q, func=AF.Ln, bias=eps_t)
            r2 = geo.tile([48, npp], dt.float16, name="r2")
            nc.scalar.activation(out=r2, in_=lnq, func=AF.Exp, scale=-0.5)
            # -sin(phi) convention: stored y*r2; W1 row = -w
            nc.vector.tensor_tensor(out=F[:, 1, 0:1024], in0=y_t[0:16, 0:1024],
                                    in1=r2[0:16, 0:1024], op=ALU.mult)
            nc.vector.tensor_tensor(out=F[:, 1, 1024:2045], in0=y_t[32:48, 4:1025],
                                    in1=r2[32:48, 4:1025], op=ALU.mult)
            xsb = geo.tile([48, npp], dt.float16, name="xsb")
            nc.scalar.activation(out=xsb, in_=xyxf[0:48, 0:npp], func=AF.Copy)
            nc.vector.tensor_tensor(out=F[:, 2, 0:1024], in0=xsb[0:16, 0:1024],
                                    in1=r2[0:16, 0:1024], op=ALU.mult)
            nc.vector.tensor_tensor(out=F[:, 2, 1024:2045], in0=xsb[32:48, 4:1025],
                                    in1=r2[32:48, 4:1025], op=ALU.mult)

        # per-sample feature DMA into the stacks (rows 64:68)
        for s in range(bpc):
            nc.sync.dma_start(out=stacks[64:68, s, :], in_=F[s:s + 1, :, :])

        # ---------------- Phase 2: MLP units ----------------
        with tc.tile_pool(name="h1_ps", bufs=2, space="PSUM") as h1_ps, \
             tc.tile_pool(name="h2_ps", bufs=2, space="PSUM") as h2_ps, \
             tc.tile_pool(name="h1r_p", bufs=4) as h1r_p:

            state = {}

            def emit_w1(ui):
                s, mi, h = units[ui]
                nv = min(1024, (ll - KOFF[mi]) - 1024 * h)
                h1t = h1_ps.tile([H, 2, 512], dt.float32, name="h1", tag="h1ps")
                for ci in range(2):
                    n = min(512, nv - 512 * ci)
                    c0 = 1024 * h + 512 * ci
                    nc.tensor.matmul(h1t[:, ci, 0:n], wd1[:, mi],
                                     stacks[:, s, c0:c0 + n], start=True, stop=True)
                h1tf = h1t.rearrange("p a b -> p (a b)")
                h1r = h1r_p.tile([H, 1024], dt.float16, name="h1r", tag="h1r")
                e = E1[ui]
                if e == "A":
                    nc.scalar.activation(out=h1r[:, 0:nv], in_=h1tf[:, 0:nv],
                                         func=AF.Relu, bias=bw[:, mi:mi + 1])
                else:
                    veng(e).tensor_scalar(out=h1r[:, 0:nv], in0=h1tf[:, 0:nv],
                                          scalar1=bw[:, mi:mi + 1], scalar2=0.0,
                                          op0=ALU.add, op1=ALU.max)
                state[ui] = (h1r, nv)

            def emit_w2(ui):
                s, mi, h = units[ui]
                h1r, nv = state.pop(ui)
                h2t = h2_ps.tile([H, 2, 512], dt.float32, name="h2", tag="h2ps")
                for ci in range(2):
                    n = min(512, nv - 512 * ci)
                    nc.tensor.matmul(h2t[:, ci, 0:n], wd2[:, mi],
                                     h1r[:, 512 * ci: 512 * ci + n],
                                     start=True, stop=True)
                h2tf = h2t.rearrange("p a b -> p (a b)")
                col = 32 * mi + 2 * s + h
                e = E2[ui]
                if e == "A":
                    nc.scalar.activation(out=scr["A"][:, 0:nv], in_=h2tf[:, 0:nv],
                                         func=AF.Relu, bias=bw[:, 3 + mi:4 + mi],
                                         accum_out=acc[:, col:col + 1])
                else:
                    veng(e).scalar_tensor_tensor(out=scr[e][:, 0:nv], in0=h2tf[:, 0:nv],
                                                 scalar=bw[:, 3 + mi:4 + mi],
                                                 in1=zeros[:, 0:nv],
                                                 op0=ALU.add, op1=ALU.max,
                                                 accum_out=acc[:, col:col + 1])

            for ui in range(len(units) + LAG):
                if ui < len(units):
                    emit_w1(ui)
                if ui >= LAG:
                    emit_w2(ui - LAG)

        # ---------------- final reduction ----------------
        with tc.tile_pool(name="fin_ps", bufs=1, space="PSUM") as fin_ps:
            ep = fin_ps.tile([1, 3, 2 * bpc], dt.float32, name="ep")
            for mi in range(3):
                nc.tensor.matmul(ep[:, mi, :], bw[:, 6 + mi:7 + mi],
                                 acc[:, 32 * mi: 32 * mi + 2 * bpc], start=True, stop=True)
            esum = consts.tile([1, bpc], dt.float32, name="esum")
            nc.vector.tensor_reduce(
                out=esum, in_=ep.rearrange("o m (s h) -> o s m h", h=2), axis=AX.XY, op=ALU.add)
            eout = consts.tile([1, bpc], dt.float32, name="eout")
            nc.vector.tensor_scalar(out=eout, in0=esum, scalar1=bw[0:1, 9:10],
                                    scalar2=None, op0=ALU.add)
            nc.sync.dma_start(out=out_d.ap(), in_=eout)

    nc.finalize()
    return nc


_NC_CACHE = {}


def get_nc(bpc=BPC, ll=L):
    key = (bpc, ll)
    if key not in _NC_CACHE:
        _NC_CACHE[key] = build_nc(bpc, ll)
    return _NC_CACHE[key]


def _sel_matrices():
    # [96, 240]: cols 0:48 = S96 sample-select (row 48h+16c+s -> col 32h+s),
    # cols 48:144 = rot1 (out p = in (c+1)%3), cols 144:240 = rot2 ((c+2)%3)
    SEL = np.zeros((96, 240), np.float16)
    for hh in range(2):
        for c in range(3):
            for i in range(16):
                SEL[48 * hh + 16 * c + i, 32 * hh + i] = 1.0
                if hh == 0:
                    # keep dummy rows 16:32 finite (Ln reads all 48 rows)
                    SEL[16 * c + i, 16 + i] = 1.0
                SEL[48 * hh + 16 * ((c + 1) % 3) + i, 48 + 48 * hh + 16 * c + i] = 1.0
                SEL[48 * hh + 16 * ((c + 2) % 3) + i, 144 + 48 * hh + 16 * c + i] = 1.0
    return SEL


def pack_weights(inputs):
    f32 = lambda k: np.asarray(inputs[k], np.float32)
    WD1 = np.zeros((KS, 3, H), np.float32)
    w = f32("fl_W1")                        # [33,H]: [len, e0, e1]
    WD1[0:32, 0] = w[1:33]
    WD1[64, 0] = w[0]                       # len-3.8 feature row
    w = f32("ft_W1")                        # [49,H]: [cos_t, e0, e1, e2]
    WD1[0:48, 1] = w[1:49]
    WD1[67, 1] = -w[0]                      # stored -cos(theta)
    w = f32("fp_W1")                        # [66,H]: [sin, cos, e0..e3]
    WD1[0:64, 2] = w[2:66]
    WD1[65, 2] = -w[0]                      # stored -sin(phi)
    WD1[66, 2] = w[1]                       # cos(phi)

    WD2 = np.zeros((H, 3, H), np.float32)
    for mi, m in enumerate(MLPS):
        WD2[:, mi] = f32(f"{m}_W2")

    BW = np.zeros((H, 10), np.float32)
    BW[:, 0] = f32("fl_b1") + 3.8 * f32("fl_W1")[0]
    BW[:, 1] = f32("ft_b1")
    BW[:, 2] = f32("fp_b1")
    for mi, m in enumerate(MLPS):
        BW[:, 3 + mi] = f32(f"{m}_b2")
        BW[:, 6 + mi] = f32(f"{m}_W3")[:, 0]
    BW[0, 9] = (float(f32("fl_b3").reshape(-1)[0]) * NL
                + float(f32("ft_b3").reshape(-1)[0]) * NT
                + float(f32("fp_b3").reshape(-1)[0]) * NP)
    return WD1.astype(np.float16), WD2.astype(np.float16), BW


def make_in_maps(inputs, bpc=BPC, ncores=NCORES):
    WD1, WD2, BW = pack_weights(inputs)
    emb16 = np.asarray(inputs["emb"], np.float32).astype(np.float16)
    seq = np.asarray(inputs["seq"], np.int64)
    R = np.asarray(inputs["R"], np.float32)
    e_all = emb16[seq]                       # [B, L, E]
    consts = dict(SEL=_sel_matrices(), WD1=WD1, WD2=WD2, BW=BW)
    in_maps = []
    for c in range(ncores):
        sl = slice(c * bpc, (c + 1) * bpc)
        Rtt = R[sl].transpose(2, 0, 1).reshape(48, L)              # [3*bpc, L]
        Rt = np.zeros((96, 1028), np.float32)
        Rt[0:48] = Rtt[:, 0:1028]
        Rt[48:96] = Rtt[:, 1020:2048]
        e = e_all[sl]                                              # [bpc, L, E]
        SE = np.zeros((bpc, 64, L), np.float16)
        for shift in range(4):
            SE[:, 16 * shift:16 * shift + 16, :L - shift] = e[:, shift:, :].transpose(0, 2, 1)
        m = dict(consts)
        m["Rt"] = Rt
        m["SE"] = SE
        in_maps.append(m)
    return in_maps


def kernel(**inputs):
    nc = get_nc()
    in_maps = make_in_maps(inputs)
    res = bass_utils.run_bass_kernel_spmd(nc, in_maps, core_ids=list(range(NCORES)))
    return np.concatenate([res.results[c]["out"][0] for c in range(NCORES)]).astype(np.float32)
